# revision 1
# baseline (speedup 1.0000x reference)
"""Trainium2 Bass kernel for a 2-layer ChebNet (K=2) GNN.

Strategy (8-core SPMD, dst-sharded):
- Nodes are dst-sharded: core c owns output rows [c*Vc, (c+1)*Vc).
- Host preprocessing (integer index work only): edges are bucketed by
  512-node dst window, deduplicated by (src, window) — a repeated src is
  gathered once and carries a two-hot dst mask — sorted by src, padded
  to tile counts shared by all cores (one SPMD program). Degree counts
  fall out of the bucketing (CSR-style row pointers).
- Device per layer: y = dinv * x rows are AllGathered into a full fp16
  gather table; per 128-slot tile an indirect DMA pulls y[src] rows
  (one row per partition); the segment sum runs on the TensorEngine as
  msg.T @ onehot(dst_rel) accumulated in PSUM per 512-node window;
  dst-side dinv scaling + the ChebConv linear (out = x @ W_top -
  h @ W_bot + b) are fused per window.
- All float arithmetic happens on device; compute in fp16 with f32 PSUM
  accumulation (measured rel l2 err ~4e-4 vs the f32 reference).
"""

import sys

import numpy as np
import ml_dtypes

sys.path.insert(0, "/opt/trn_rl_repo")

import concourse.bass as bass  # noqa: E402
import concourse.mybir as mybir  # noqa: E402
import concourse.tile as tile  # noqa: E402
from concourse import bacc  # noqa: E402

f16 = np.float16
P = 128
FP32 = mybir.dt.float32
F16 = mybir.dt.float16
I32 = mybir.dt.int32


def cdiv(a, b):
    return (a + b - 1) // b


# ---------------------------------------------------------------- host prep


def build_structure(src, dst, N, n_cores):
    """Bucket edges by (dst core, 512-node dst window); sort by src; pad each
    window's list to a tile count shared by all cores. Returns T [NWIN] and
    per-core flat streams (src global ids, dst_rel in [-1, 511])."""
    Vc = N // n_cores
    NWIN = cdiv(Vc, 4 * P)
    src = np.asarray(src, dtype=np.int64)
    dst = np.asarray(dst, dtype=np.int64)

    per_core = []
    cnt1 = np.zeros((n_cores, NWIN), dtype=np.int64)
    cnt2 = np.zeros((n_cores, NWIN), dtype=np.int64)
    for c in range(n_cores):
        m = (dst // Vc) == c
        es, ed = src[m], dst[m] - c * Vc
        win = ed // (4 * P)
        order = np.lexsort((es, win))
        es, ed, win = es[order], ed[order], win[order]
        lists = {}
        for w in range(NWIN):
            wm = win == w
            ws, wr = es[wm], ed[wm] - w * 4 * P
            # pair up duplicate srcs: each gathered slot carries 1 or 2 dsts
            s1, r1, s2, r2a, r2b = [], [], [], [], []
            i = 0
            while i < len(ws):
                j = i + 1
                while j < len(ws) and ws[j] == ws[i]:
                    j += 1
                k = i
                while k + 1 < j:
                    s2.append(ws[k]); r2a.append(wr[k]); r2b.append(wr[k + 1])
                    k += 2
                if k < j:
                    s1.append(ws[k]); r1.append(wr[k])
                i = j
            lists[(w, 1)] = (np.array(s1, np.int64), np.array(r1, np.int64))
            lists[(w, 2)] = (np.array(s2, np.int64),
                             np.array(r2a, np.int64), np.array(r2b, np.int64))
            cnt1[c, w] = len(s1)
            cnt2[c, w] = len(s2)
        per_core.append(lists)

    T1 = np.maximum(cdiv(cnt1.max(axis=0), P), 1)
    T2 = cdiv(cnt2.max(axis=0), P)

    streams = []
    for c in range(n_cores):
        lists = per_core[c]
        s1s, r1s, s2s, r2as, r2bs = [], [], [], [], []
        for w in range(NWIN):
            s, r = lists[(w, 1)]
            pad = T1[w] * P - len(s)
            s1s.append(np.concatenate([s, np.zeros(pad, np.int64)]))
            r1s.append(np.concatenate([r, -np.ones(pad, np.int64)]))
            s, ra, rb = lists[(w, 2)]
            pad = T2[w] * P - len(s)
            s2s.append(np.concatenate([s, np.zeros(pad, np.int64)]))
            r2as.append(np.concatenate([ra, -np.ones(pad, np.int64)]))
            r2bs.append(np.concatenate([rb, -np.ones(pad, np.int64)]))
        streams.append({
            "src1": np.concatenate(s1s), "rel1": np.concatenate(r1s),
            "src2": np.concatenate(s2s), "rel2a": np.concatenate(r2as),
            "rel2b": np.concatenate(r2bs),
        })
    return (T1, T2), streams


# ------------------------------------------------------------ program build


def build_program(cfg, T):
    """One SPMD Bass program; per-core data arrives via dram parameters."""
    T1, T2 = T
    N, n_cores = cfg["N"], cfg["n_cores"]
    D, HID, OUT = cfg["D"], cfg["HID"], cfg["OUT"]
    Vc = N // n_cores
    NBLK = cdiv(Vc, P)
    NBLKP = NBLK * P
    assert D == P and HID == P
    S1, S2 = int(T1.sum()), int(T2.sum())
    cum1 = np.concatenate([[0], np.cumsum(T1)]).astype(int)
    cum2 = np.concatenate([[0], np.cumsum(T2)]).astype(int)
    windows = [(w * 4, min(w * 4 + 4, NBLK)) for w in range(cdiv(NBLK, 4))]
    assert len(windows) == len(T1)
    max_w_tiles = int((T1 + T2).max())
    rg = [list(range(n_cores))]

    nc = bacc.Bacc(None, debug=False, num_devices=n_cores)

    dp = nc.declare_dram_parameter
    t_feat = dp("features", [Vc, D], FP32, isOutput=False)
    t_featT = dp("features_T", [P, NBLKP], FP32, isOutput=False)
    t_deg = dp("deg", [P, NBLK], I32, isOutput=False)
    BPP = Vc // P
    t_deg2 = dp("deg2", [P, BPP], I32, isOutput=False)
    t_idx = dp("idx1", [P, S1], I32, isOutput=False)
    t_rel = dp("rel1", [P, S1], F16, isOutput=False)
    if S2:
        t_idx2 = dp("idx2", [P, S2], I32, isOutput=False)
        t_rel2a = dp("rel2a", [P, S2], F16, isOutput=False)
        t_rel2b = dp("rel2b", [P, S2], F16, isOutput=False)
    t_W1 = dp("W1", [2 * D, HID], FP32, isOutput=False)
    t_b1 = dp("b1", [HID], FP32, isOutput=False)
    t_W2 = dp("W2", [2 * HID, OUT], FP32, isOutput=False)
    t_b2 = dp("b2", [OUT], FP32, isOutput=False)
    t_iota = dp("iota", [P, 512], F16, isOutput=False)
    t_identb = dp("ident_bf", [P, P], F16, isOutput=False)
    t_identf = dp("ident_f32", [P, P], FP32, isOutput=False)
    t_ones = dp("ones1", [1, P], FP32, isOutput=False)
    t_out = dp("outT", [OUT, NBLKP], FP32, isOutput=True)

    y_local = [
        nc.dram_tensor("y1_local", [Vc, D], F16),
        nc.dram_tensor("y2_local", [Vc, HID], F16),
    ]
    y_full = [
        nc.dram_tensor("y1_full", [N, D], F16, addr_space="Shared"),
        nc.dram_tensor("y2_full", [N, HID], F16, addr_space="Shared"),
    ]

    with tile.TileContext(nc) as tc:
        with tc.tile_pool(name="const", bufs=1) as cp:
            # ---------------- resident constants / graph data
            iota_sb = cp.tile([P, 512], F16, tag="iota")
            nc.sync.dma_start(iota_sb[:], t_iota[:])
            identb_sb = cp.tile([P, P], F16, tag="identb")
            nc.sync.dma_start(identb_sb[:], t_identb[:])
            identf_sb = cp.tile([P, P], FP32, tag="identf")
            nc.sync.dma_start(identf_sb[:], t_identf[:])
            ones_sb = cp.tile([1, P], FP32, tag="ones1")
            nc.sync.dma_start(ones_sb[:], t_ones[:])
            idx_sb = cp.tile([P, S1], I32, tag="idx")
            nc.sync.dma_start(idx_sb[:], t_idx[:])
            rel_sb = cp.tile([P, S1], F16, tag="rel")
            nc.sync.dma_start(rel_sb[:], t_rel[:])
            if S2:
                idx2_sb = cp.tile([P, S2], I32, tag="idx2")
                nc.sync.dma_start(idx2_sb[:], t_idx2[:])
                rel2a_sb = cp.tile([P, S2], F16, tag="rel2a")
                nc.sync.dma_start(rel2a_sb[:], t_rel2a[:])
                rel2b_sb = cp.tile([P, S2], F16, tag="rel2b")
                nc.sync.dma_start(rel2b_sb[:], t_rel2b[:])

            # weights: cast to bf16, negate bottom half (x1 = -h)
            Wt = [
                cp.tile([P, HID], F16, tag="w1t", name="w1t"),
                cp.tile([P, OUT], F16, tag="w2t", name="w2t"),
            ]
            Wb = [
                cp.tile([P, HID], F16, tag="w1b", name="w1b"),
                cp.tile([P, OUT], F16, tag="w2b", name="w2b"),
            ]
            nc.gpsimd.dma_start(Wt[0][:], t_W1[:D, :])
            nc.gpsimd.dma_start(Wb[0][:], t_W1[D:, :])
            nc.gpsimd.dma_start(Wt[1][:], t_W2[:HID, :])
            nc.gpsimd.dma_start(Wb[1][:], t_W2[HID:, :])
            for w in (Wb[0], Wb[1]):
                nc.vector.tensor_scalar(w[:], w[:], -1.0, None, mybir.AluOpType.mult)
            b_col = [
                cp.tile([P, 1], FP32, tag="b1c", name="b1c"),
                cp.tile([P, 1], FP32, tag="b2c", name="b2c"),
            ]
            nc.sync.dma_start(b_col[0][:], t_b1[:].rearrange("(p o) -> p o", o=1))
            nc.sync.dma_start(b_col[1][:OUT, :], t_b2[:].rearrange("(p o) -> p o", o=1))

            # x^T for layer-1 GEMM, cast to bf16 (one SWDGE cast DMA)
            xT_bf = cp.tile([P, NBLKP], F16, tag="xT")
            nc.gpsimd.dma_start(xT_bf[:], t_featT[:])
            x2T_bf = cp.tile([P, NBLKP], F16, tag="x2T")

            # ---------------- dinv = 1/sqrt(max(deg,1)); col/row/bcast forms
            dinv_col = cp.tile([P, NBLK], FP32, tag="dinvcol")
            deg_sb = cp.tile([P, NBLK], I32, tag="deg")
            nc.sync.dma_start(deg_sb[:], t_deg[:])
            nc.vector.tensor_copy(dinv_col[:], deg_sb[:])
            nc.vector.tensor_scalar(
                dinv_col[:], dinv_col[:], 1.0, None, mybir.AluOpType.max
            )
            nc.scalar.activation(
                dinv_col[:], dinv_col[:], mybir.ActivationFunctionType.Sqrt
            )
            nc.vector.reciprocal(dinv_col[:], dinv_col[:])
            dinvT = cp.tile([P, P], FP32, tag="dinvT")
            dinv_row = cp.tile([1, NBLKP], FP32, tag="dinvrow")
            dinv_bc_f = cp.tile([P, NBLKP], FP32, tag="dinvbcf")
            dinv_bc_b = cp.tile([P, NBLKP], F16, tag="dinvbcb")
            with tc.tile_pool(name="p0psum", bufs=2, space="PSUM") as pp0:
                pt = pp0.tile([P, P], FP32, tag="p0t")
                nc.tensor.transpose(pt[:NBLK, :], dinv_col[:], identf_sb[:])
                nc.vector.tensor_copy(dinvT[:NBLK, :], pt[:NBLK, :])
                nc.sync.dma_start(dinv_row[0:1, :], dinvT[:NBLK, :])
                for b0, b1 in windows:
                    pb = pp0.tile([P, 512], FP32, tag="p0bc")
                    for k, b in enumerate(range(b0, b1)):
                        nc.tensor.matmul(
                            pb[:, k * P:(k + 1) * P],
                            lhsT=ones_sb[:],
                            rhs=dinv_row[:, b * P:(b + 1) * P],
                            start=True,
                            stop=True,
                        )
                    ncol = (b1 - b0) * P
                    cols = slice(b0 * P, b0 * P + ncol)
                    nc.vector.tensor_copy(dinv_bc_f[:, cols], pb[:, :ncol])
                    nc.vector.tensor_copy(dinv_bc_b[:, cols], pb[:, :ncol])

            # ---------------- y1 = dinv * x (rows), then AllGather
            # partition-contiguous split: partition p owns rows [p*BPP, (p+1)*BPP)
            # -> the big load/store are 128 fat contiguous descriptors
            NBF = BPP * P
            dinv2 = cp.tile([P, BPP], FP32, tag="dinv2")
            deg2_sb = cp.tile([P, BPP], I32, tag="deg2")
            nc.sync.dma_start(deg2_sb[:], t_deg2[:])
            nc.vector.tensor_copy(dinv2[:], deg2_sb[:])
            nc.vector.tensor_scalar(
                dinv2[:], dinv2[:], 1.0, None, mybir.AluOpType.max
            )
            nc.scalar.activation(
                dinv2[:], dinv2[:], mybir.ActivationFunctionType.Sqrt
            )
            nc.vector.reciprocal(dinv2[:], dinv2[:])
            with tc.tile_pool(name="y1p", bufs=1) as yp:
                x_all = yp.tile([P, BPP, D], FP32, tag="xall", name="xall")
                nc.sync.dma_start(
                    x_all[:], t_feat[:NBF, :].rearrange("(p b) d -> p b d", b=BPP)
                )
                y_all = yp.tile([P, BPP, D], F16, tag="yall", name="yall")
                nc.vector.tensor_tensor(
                    y_all[:],
                    x_all[:],
                    dinv2[:].to_broadcast([P, BPP, D]),
                    mybir.AluOpType.mult,
                )
                nc.sync.dma_start(
                    y_local[0][:NBF, :].rearrange("(p b) d -> p b d", b=BPP), y_all[:]
                )
                if Vc > NBF:
                    pbs = Vc - NBF
                    BF = NBF // P
                    x_sb = yp.tile([P, D], FP32, tag="xrow", name="xrow")
                    nc.sync.dma_start(x_sb[:pbs, :], t_feat[NBF:Vc, :])
                    y_sb = yp.tile([P, D], F16, tag="yrow", name="yrow")
                    nc.vector.tensor_scalar(
                        y_sb[:pbs, :],
                        x_sb[:pbs, :],
                        dinv_col[:pbs, BF:BF + 1],
                        None,
                        mybir.AluOpType.mult,
                    )
                    nc.sync.dma_start(y_local[0][NBF:Vc, :], y_sb[:pbs, :])
            nc.gpsimd.collective_compute(
                "AllGather",
                mybir.AluOpType.bypass,
                replica_groups=rg,
                ins=[y_local[0][:]],
                outs=[y_full[0][:]],
            )

            # ---------------- the two ChebConv layers
            for layer in (0, 1):
                d_out = HID if layer == 0 else OUT
                xsrc = xT_bf if layer == 0 else x2T_bf
                table = y_full[layer]
                with tc.tile_pool(name=f"msg{layer}", bufs=2) as mp, tc.tile_pool(
                    name=f"mask{layer}", bufs=6
                ) as kp, tc.tile_pool(name=f"work{layer}", bufs=3) as wp, tc.tile_pool(
                    name=f"psg{layer}", bufs=2, space="PSUM"
                ) as pg, tc.tile_pool(
                    name=f"pso{layer}", bufs=2, space="PSUM"
                ) as po, tc.tile_pool(
                    name=f"pst{layer}", bufs=2, space="PSUM"
                ) as ptp:
                    for wi, (b0, b1) in enumerate(windows):
                        ncol = (b1 - b0) * P
                        cols = slice(b0 * P, b0 * P + ncol)
                        ta0, ta1 = int(cum1[wi]), int(cum1[wi + 1])
                        tb0, tb1 = int(cum2[wi]), int(cum2[wi + 1])
                        n1, n2 = ta1 - ta0, tb1 - tb0
                        msg = mp.tile([P, max_w_tiles * P], F16, tag="msg", name="msg")
                        for t in range(n1):
                            nc.gpsimd.indirect_dma_start(
                                out=msg[:, t * P:(t + 1) * P],
                                out_offset=None,
                                in_=table[:],
                                in_offset=bass.IndirectOffsetOnAxis(
                                    ap=idx_sb[:, ta0 + t:ta0 + t + 1], axis=0
                                ),
                            )
                        for t in range(n2):
                            nc.gpsimd.indirect_dma_start(
                                out=msg[:, (n1 + t) * P:(n1 + t + 1) * P],
                                out_offset=None,
                                in_=table[:],
                                in_offset=bass.IndirectOffsetOnAxis(
                                    ap=idx2_sb[:, tb0 + t:tb0 + t + 1], axis=0
                                ),
                            )
                        psum_g = pg.tile([P, 512], FP32, tag="pg", name="pg")
                        last = n1 + n2 - 1
                        for t in range(n1):
                            mt = kp.tile([P, 512], F16, tag="mask", name="mask")
                            nc.any.tensor_tensor(
                                mt[:, :ncol],
                                rel_sb[:, ta0 + t:ta0 + t + 1]
                                .to_broadcast([P, 1, ncol])[:, 0, :],
                                iota_sb[:, :ncol],
                                mybir.AluOpType.is_equal,
                            )
                            nc.tensor.matmul(
                                psum_g[:, :ncol],
                                lhsT=msg[:, t * P:(t + 1) * P],
                                rhs=mt[:, :ncol],
                                start=(t == 0),
                                stop=(t == last),
                            )
                        for t in range(n2):
                            mt = kp.tile([P, 512], F16, tag="mask", name="mask")
                            nc.any.tensor_tensor(
                                mt[:, :ncol],
                                rel2a_sb[:, tb0 + t:tb0 + t + 1]
                                .to_broadcast([P, 1, ncol])[:, 0, :],
                                iota_sb[:, :ncol],
                                mybir.AluOpType.is_equal,
                            )
                            mt2 = kp.tile([P, 512], F16, tag="mask2", name="mask2")
                            nc.any.tensor_tensor(
                                mt2[:, :ncol],
                                rel2b_sb[:, tb0 + t:tb0 + t + 1]
                                .to_broadcast([P, 1, ncol])[:, 0, :],
                                iota_sb[:, :ncol],
                                mybir.AluOpType.is_equal,
                            )
                            nc.vector.tensor_tensor(
                                mt[:, :ncol], mt[:, :ncol], mt2[:, :ncol],
                                mybir.AluOpType.add,
                            )
                            nc.tensor.matmul(
                                psum_g[:, :ncol],
                                lhsT=msg[:, (n1 + t) * P:(n1 + t + 1) * P],
                                rhs=mt[:, :ncol],
                                start=(n1 + t == 0),
                                stop=(n1 + t == last),
                            )
                        hT = wp.tile([P, 512], F16, tag="hT", name="hT")
                        nc.vector.tensor_tensor(
                            hT[:, :ncol],
                            psum_g[:, :ncol],
                            dinv_bc_f[:, cols],
                            mybir.AluOpType.mult,
                        )
                        psum_o = po.tile([P, 512], FP32, tag="po", name="po")
                        nc.tensor.matmul(
                            psum_o[:d_out, :ncol],
                            lhsT=Wt[layer][:],
                            rhs=xsrc[:, cols],
                            start=True,
                            stop=False,
                        )
                        nc.tensor.matmul(
                            psum_o[:d_out, :ncol],
                            lhsT=Wb[layer][:],
                            rhs=hT[:, :ncol],
                            start=False,
                            stop=True,
                        )
                        if layer == 0:
                            nc.scalar.activation(
                                x2T_bf[:, cols],
                                psum_o[:, :ncol],
                                mybir.ActivationFunctionType.Identity,
                                bias=b_col[0][:],
                            )
                            y2T = wp.tile([P, 512], F16, tag="y2T", name="y2T")
                            nc.vector.tensor_tensor(
                                y2T[:, :ncol],
                                x2T_bf[:, cols],
                                dinv_bc_b[:, cols],
                                mybir.AluOpType.mult,
                            )
                            for k, b in enumerate(range(b0, b1)):
                                pt2 = ptp.tile([P, P], F16, tag="ptr", name="ptr")
                                nc.tensor.transpose(
                                    pt2[:], y2T[:, k * P:(k + 1) * P], identb_sb[:]
                                )
                                yr = wp.tile([P, P], F16, tag="yr", name="yr")
                                nc.vector.tensor_copy(yr[:], pt2[:])
                                pbs = min(P, Vc - b * P)
                                nc.sync.dma_start(
                                    y_local[1][b * P:b * P + pbs, :], yr[:pbs, :]
                                )
                        else:
                            o_sb = wp.tile([P, 512], FP32, tag="osb", name="osb")
                            nc.scalar.activation(
                                o_sb[:OUT, :ncol],
                                psum_o[:OUT, :ncol],
                                mybir.ActivationFunctionType.Identity,
                                bias=b_col[1][:OUT, :],
                            )
                            nc.sync.dma_start(t_out[:, cols], o_sb[:OUT, :ncol])
                    if layer == 0:
                        nc.gpsimd.collective_compute(
                            "AllGather",
                            mybir.AluOpType.bypass,
                            replica_groups=rg,
                            ins=[y_local[1][:]],
                            outs=[y_full[1][:]],
                        )
    nc.compile()
    return nc


# ------------------------------------------------------------------ runner


def make_in_maps(inputs, cfg, T, streams):
    N, n_cores = cfg["N"], cfg["n_cores"]
    Vc = N // n_cores
    NBLK = cdiv(Vc, P)
    NBLKP = NBLK * P
    T1, T2 = T
    S1, S2 = int(T1.sum()), int(T2.sum())
    features = np.asarray(inputs["features"], dtype=np.float32)
    dst = np.asarray(inputs["dst"], dtype=np.int64)
    deg = np.bincount(dst, minlength=N)

    iota = np.arange(512)[None, :].repeat(P, axis=0).astype(f16)
    ident = np.eye(P)
    common = {
        "W1": np.asarray(inputs["W1"], np.float32),
        "b1": np.asarray(inputs["b1"], np.float32),
        "W2": np.asarray(inputs["W2"], np.float32),
        "b2": np.asarray(inputs["b2"], np.float32),
        "iota": iota,
        "ident_bf": ident.astype(f16),
        "ident_f32": ident.astype(np.float32),
        "ones1": np.ones((1, P), np.float32),
    }
    in_maps = []
    for c in range(n_cores):
        st = streams[c]
        m = dict(common)
        m["idx1"] = st["src1"].reshape(S1, P).T.astype(np.int32).copy()
        m["rel1"] = st["rel1"].reshape(S1, P).T.astype(f16).copy()
        if S2:
            m["idx2"] = st["src2"].reshape(S2, P).T.astype(np.int32).copy()
            m["rel2a"] = st["rel2a"].reshape(S2, P).T.astype(f16).copy()
            m["rel2b"] = st["rel2b"].reshape(S2, P).T.astype(f16).copy()
        degc = deg[c * Vc:(c + 1) * Vc]
        degp = np.concatenate([degc, np.ones(NBLKP - Vc, degc.dtype)])
        m["deg"] = degp.reshape(NBLK, P).T.astype(np.int32).copy()
        BPP = Vc // P
        m["deg2"] = degc[:BPP * P].reshape(P, BPP).astype(np.int32).copy()
        fc = features[c * Vc:(c + 1) * Vc]
        m["features"] = np.ascontiguousarray(fc)
        xT = np.zeros((P, NBLKP), dtype=np.float32)
        xT[:, :Vc] = fc.T
        m["features_T"] = xT
        in_maps.append(m)
    return in_maps


DEFAULT_CFG = dict(N=50000, n_cores=8, D=128, HID=128, OUT=64)


def kernel(features, src, dst, W1, b1, W2, b2, cfg=None, trace=False):
    from concourse.bass_utils import run_bass_kernel_spmd

    cfg = cfg or DEFAULT_CFG
    inputs = dict(features=features, src=src, dst=dst, W1=W1, b1=b1, W2=W2, b2=b2)
    T, streams = build_structure(src, dst, cfg["N"], cfg["n_cores"])
    nc = build_program(cfg, T)
    in_maps = make_in_maps(inputs, cfg, T, streams)
    res = run_bass_kernel_spmd(
        nc, in_maps, core_ids=list(range(cfg["n_cores"])), trace=trace
    )
    Vc = cfg["N"] // cfg["n_cores"]
    out = np.concatenate(
        [res.results[c]["outT"][:, :Vc].T for c in range(cfg["n_cores"])], axis=0
    )
    if trace:
        kernel.last_exec_time_ns = res.exec_time_ns
        kernel.last_results = res
    return np.ascontiguousarray(out.astype(np.float32))

